# revision 5
# baseline (speedup 1.0000x reference)
"""DeepSSM Trainium2 kernel v2 (8 NeuronCores, data-parallel over batch).

Same math as v1 (see kernel.py docstring for the conv-collapse derivation)
but restructured around the cost model:

- Stream state is the NORMALIZED activations t (post-LN), kept fp16 in
  SBUF.  This removes v1's phase-A pass entirely: gelu reads t directly
  with the per-partition Dp scale.
- All big elementwise DVE ops use fp16 operands in SBUF, which the DVE
  runs in 4x mode (TensorScalarPtr) or 2x mode (TensorTensor).
- The proj matmuls run in fp16 (1 cyc/row at any N, vs fp32r needing
  N>=256).
- LN variance is computed by 32 tiny fp16 matmuls (ones^T @ sq per
  128-position sub-chunk) writing each sub-chunk's row to its own PSUM
  partition, so var lands already "transposed" as S[16,128] — no DMA
  reshape round-trip.  rsqrt = 5-op bit-trick (1 Newton step) on
  [16,128], then 16 tiny selector matmuls broadcast inv back to
  [128,512] PSUM tiles per chunk.
- The two PSUM-reading joins (Y = ps+pbt+t and t' = Y*inv) are split
  between DVE-direct (fp32-rate STT) and ACT-evac (+per-partition bias)
  followed by a 4x fp16 STT, to balance DVE vs ACT load.
"""

import numpy as np

D_MODEL = 256
N_LAYERS = 8
NUM_CLASSES = 3
BATCH = 8
SEQ = 2048
JW = 256
TAIL = 256
LN_EPS = 1e-5
EXIT_LAYERS = (1, 3, 5, 7)
NT = 4
NTW = SEQ // NT  # 512
NSUB = 16  # 128-position sub-chunks

_CACHE = {}

# engine assignment config (sweepable): per-nt engines
CONFIG = dict(
    join_eng=("d", "d", "a", "a"),   # Y-evac: d=DVE TS, a=ACT Identity
    tp_eng=("d", "d", "d", "d"),     # t' TT engine: d=DVE, p=Pool
    sq_eng=("d", "d", "d", "d"),     # per-nt: a=ACT Square(ps+pbt) from PSUM, d=DVE TT
    gelu_pieces=((0, 512), (512, 1024), (1024, 1792)),
    rsqrt_pairs=True,                 # split rsqrt into two nt-pair halves
    pw_bufs=4,
    pool_pooled=False,                 # exit-layer pooled pass on Pool (last on DVE)
)


def _host_prep(inputs):
    f64 = np.float64
    A = 1.0 / (1.0 + np.exp(-inputs["A_params"].astype(f64)))  # [nl, d]
    lnA = np.log(A)
    CB = inputs["C_params"].astype(f64) * inputs["B_params"].astype(f64)
    j1 = np.arange(JW, dtype=f64)
    lt = (TAIL - 1.0) - np.arange(TAIL, dtype=f64)
    W1 = np.exp(lnA[:, :, None] * j1[None, None, :])            # [nl, d, JW]
    Wt = CB[:, :, None] * np.exp(lnA[:, :, None] * lt[None, None, :])

    def to_chunks(T, dt):  # [nl, d, l] -> [128, nl, 2, l]
        return np.ascontiguousarray(
            T.reshape(N_LAYERS, 2, 128, -1).transpose(2, 0, 1, 3)
        ).astype(dt)

    pW = inputs["proj_W"].astype(f64)                            # [nl, do, di]
    pWc = pW - pW.mean(axis=1, keepdims=True)
    PtT_all = np.ascontiguousarray(
        pWc.transpose(0, 2, 1).reshape(N_LAYERS, 2, 128, D_MODEL).transpose(2, 0, 1, 3)
    ).astype(np.float16)                                          # [128,nl,2,256]

    Dp_all = np.ascontiguousarray(
        inputs["D_params"].reshape(N_LAYERS, 2, 128).transpose(2, 0, 1)
    ).astype(np.float32)
    pb = inputs["proj_b"].astype(f64)
    pbt = pb - pb.mean(axis=1, keepdims=True)
    pbt_all = np.ascontiguousarray(
        pbt.reshape(N_LAYERS, 2, 128).transpose(2, 0, 1)
    ).astype(np.float32)

    # layer-0 stream is RAW h0 = inW*x + in_b; its centering for the LN
    # stats rides in via corrW (x-dependent) and the pbt[0] constant.
    inW = inputs["in_W"][:, 0].astype(f64)
    inb = inputs["in_b"].astype(f64)
    corrW_row = np.full((1, D_MODEL), -inW.mean(), dtype=np.float16)
    pbt_all[:, 0, :] -= np.float32(inb.mean())

    hW = inputs["head_W"].astype(f64) / SEQ                      # [4, nc, d]
    headWT_all = np.ascontiguousarray(
        hW.transpose(2, 0, 1).reshape(2, 128, 4, NUM_CLASSES).transpose(1, 0, 2, 3)
    ).astype(np.float32)                                          # [128,2,4,3]
    headb_all = np.ascontiguousarray(
        inputs["head_b"].astype(np.float32).T.reshape(NUM_CLASSES, 4)
    )

    sel = np.zeros((8, NSUB * 128), np.float16)
    for g in range(NSUB):
        sel[g % 8, g * 128:(g + 1) * 128] = 1.0


    return dict(
        W1_all=to_chunks(W1, np.float16),
        Wt_all=to_chunks(Wt, np.float16),
        PtT_all=PtT_all,
        Dp_all=Dp_all,
        pbt_all=pbt_all,
        corrW_row=corrW_row,
        sel_all=sel,
        ident_in=np.ascontiguousarray(np.eye(128, dtype=np.float16)),
        onesI_in=np.ascontiguousarray(
            np.tile(np.eye(8, dtype=np.float16)[None] / D_MODEL, (128, 1, 1))
        ),
        headWT_all=headWT_all,
        headb_all=headb_all,
    )


def _split_drain_waits(nc, mybir, maxw=1):
    """Hoist excess sync waits onto same-engine NOPs (walrus ISA limit)."""
    for f in nc.m.functions:
        for blk in f.blocks:
            insts = list(blk.instructions)
            changed = False
            new_list = []
            for ins in insts:
                w = (
                    list(ins.sync_info.on_wait)
                    if ins.sync_info and ins.sync_info.on_wait
                    else []
                )
                if len(w) > maxw:
                    changed = True
                    extra, keep = w[:-maxw], w[-maxw:]
                    for j in range(0, len(extra), maxw):
                        nop = mybir.InstNoOp(
                            name=f"{ins.name}-wsplit{j}", ins=[], outs=[]
                        )
                        nop.engine = ins.engine
                        nop.sync_info = mybir.SyncInfo(
                            on_wait=extra[j : j + maxw], on_update=[]
                        )
                        new_list.append(nop)
                    ins.sync_info.on_wait = keep
                new_list.append(ins)
            if changed:
                blk.instructions = new_list


def _build_nc(sim_safe=False, split=True):
    import concourse.bass as bass
    import concourse.tile as tile
    import concourse.mybir as mybir

    F32 = mybir.dt.float32
    F32R = mybir.dt.float32r
    F16 = mybir.dt.float16
    I32 = mybir.dt.int32
    OP = mybir.AluOpType
    ACTF = mybir.ActivationFunctionType
    GELU = ACTF.Sigmoid if sim_safe else ACTF.Gelu

    nc = bass.Bass("TRN2", target_bir_lowering=False, debug=False)

    d_h0 = nc.dram_tensor("h0_in", [128, 2, SEQ], F16, kind="ExternalInput")
    d_W1 = nc.dram_tensor("W1_all", [128, N_LAYERS, 2, JW], F16, kind="ExternalInput")
    d_Wt = nc.dram_tensor("Wt_all", [128, N_LAYERS, 2, TAIL], F16, kind="ExternalInput")
    d_Pt = nc.dram_tensor("PtT_all", [128, N_LAYERS, 2, D_MODEL], F16, kind="ExternalInput")
    d_Dp = nc.dram_tensor("Dp_all", [128, N_LAYERS, 2], F32, kind="ExternalInput")
    d_pbt = nc.dram_tensor("pbt_all", [128, N_LAYERS, 2], F32, kind="ExternalInput")
    d_corrW = nc.dram_tensor("corrW_row", [1, D_MODEL], F16, kind="ExternalInput")
    d_x16 = nc.dram_tensor("x_row16", [1, SEQ], F16, kind="ExternalInput")
    d_sel = nc.dram_tensor("sel_all", [8, NSUB * 128], F16, kind="ExternalInput")
    d_onesI = nc.dram_tensor("onesI_in", [128, 8, 8], F16, kind="ExternalInput")
    d_ident = nc.dram_tensor("ident_in", [128, 128], F16, kind="ExternalInput")
    d_hW = nc.dram_tensor("headWT_all", [128, 2, 4, NUM_CLASSES], F32, kind="ExternalInput")
    d_hb = nc.dram_tensor("headb_all", [NUM_CLASSES, 4], F32, kind="ExternalInput")
    d_out = nc.dram_tensor("logits_out", [NUM_CLASSES, 4], F32, kind="ExternalOutput")

    with tile.TileContext(nc) as tc:
        from contextlib import ExitStack

        ctx = ExitStack()
        with ctx:
            const = ctx.enter_context(tc.tile_pool(name="const", bufs=1))
            stream = ctx.enter_context(tc.tile_pool(name="stream", bufs=CONFIG.get("stream_bufs", 3)))
            ypool = ctx.enter_context(tc.tile_pool(name="ypool", bufs=CONFIG.get("y_bufs", 2)))
            vpool = ctx.enter_context(tc.tile_pool(name="vpool", bufs=CONFIG.get("v_bufs", 2)))
            sqpool = ctx.enter_context(tc.tile_pool(name="sqpool", bufs=CONFIG.get("sq_bufs", 2)))
            epool = ctx.enter_context(tc.tile_pool(name="epool", bufs=2))
            small = ctx.enter_context(tc.tile_pool(name="small", bufs=2))
            stat = ctx.enter_context(tc.tile_pool(name="stat", bufs=2))
            pacc = ctx.enter_context(tc.tile_pool(name="pacc", bufs=5))
            pw = ctx.enter_context(tc.tile_pool(name="pw", bufs=CONFIG.get("pw_bufs", 3), space="PSUM"))
            pstat = ctx.enter_context(tc.tile_pool(name="pstat", bufs=1, space="PSUM"))
            pinv = ctx.enter_context(tc.tile_pool(name="pinv", bufs=2, space="PSUM"))

            # ---- constants / weights to SBUF ----
            # SP issues DMAs in-order: layer-0's working set first, cold
            # layers and head tables last.
            t = stream.tile([128, 2, SEQ], F16, tag="stream")
            nc.sync.dma_start(out=t[:, :, 0:NTW], in_=d_h0.ap()[:, :, 0:NTW])
            Dp_sb = const.tile([128, N_LAYERS, 2], F32)
            nc.sync.dma_start(out=Dp_sb[:], in_=d_Dp.ap())
            ident_sb = const.tile([128, 128], F16)
            nc.sync.dma_start(out=ident_sb[:], in_=d_ident.ap())
            Pt_sb = const.tile([128, N_LAYERS, 2, D_MODEL], F16)
            W1_sb = const.tile([128, N_LAYERS, 2, JW], F16)
            Wt_sb = const.tile([128, N_LAYERS, 2, TAIL], F16)
            nc.sync.dma_start(out=W1_sb[:, 0], in_=d_W1.ap()[:, 0])
            nc.sync.dma_start(out=t[:, :, NTW : 2 * NTW], in_=d_h0.ap()[:, :, NTW : 2 * NTW])
            nc.sync.dma_start(out=Wt_sb[:, 0], in_=d_Wt.ap()[:, 0])
            nc.sync.dma_start(out=Pt_sb[:, 0], in_=d_Pt.ap()[:, 0])
            nc.sync.dma_start(out=t[:, :, 2 * NTW : 3 * NTW], in_=d_h0.ap()[:, :, 2 * NTW : 3 * NTW])
            nc.sync.dma_start(out=t[:, :, 3 * NTW :], in_=d_h0.ap()[:, :, 3 * NTW :])
            pbt_sb = const.tile([128, N_LAYERS, 2], F32)
            nc.sync.dma_start(out=pbt_sb[:], in_=d_pbt.ap())
            x16_row = const.tile([1, SEQ], F16)
            nc.sync.dma_start(out=x16_row[:], in_=d_x16.ap())
            corrW_row = const.tile([1, D_MODEL], F16)
            nc.sync.dma_start(out=corrW_row[:], in_=d_corrW.ap())
            onesI_sb = const.tile([128, 8, 8], F16)
            nc.sync.dma_start(out=onesI_sb[:], in_=d_onesI.ap())
            sel_sb = const.tile([8, NSUB * 128], F16)
            nc.sync.dma_start(out=sel_sb[:], in_=d_sel.ap())
            for i in range(1, N_LAYERS):
                nc.sync.dma_start(out=W1_sb[:, i], in_=d_W1.ap()[:, i])
                nc.sync.dma_start(out=Wt_sb[:, i], in_=d_Wt.ap()[:, i])
                nc.sync.dma_start(out=Pt_sb[:, i], in_=d_Pt.ap()[:, i])
            hW_sb = const.tile([128, 2, 4, NUM_CLASSES], F32)
            nc.sync.dma_start(out=hW_sb[:], in_=d_hW.ap())
            hb_sb = const.tile([NUM_CLASSES, 4], F32)
            nc.sync.dma_start(out=hb_sb[:], in_=d_hb.ap())
            c_one = const.tile([NSUB, 128], I32)
            nc.vector.memset(c_one, 1)
            eps_col = const.tile([8, 1], F32)
            nc.vector.memset(eps_col, LN_EPS)
            c_magic = const.tile([NSUB, 128], I32)
            nc.vector.memset(c_magic, 0x5F3759DF)

            logits_sb = const.tile([NUM_CLASSES, 4], F32)
            pooled = {}

            # ---------------- shared tail: Y -> t_next ----------------
            cfg = CONFIG
            JOIN_ACT = tuple(e == "a" for e in cfg["join_eng"])

            def tail(Y, sq, li, exit_idx):
                """Y fp16 [128,2,SEQ] (centered pre-LN) -> t fp16 (normalized).

                Emitted per nt-chunk (sq+stats right after the producing
                joins), with the rsqrt split into two nt-pair halves so the
                second half's statistics overlap the first half's broadcast
                and t' work."""
                # two [8,128] stat tiles: HW requires partition offset 0 (or
                # 32/64) on every access, so the pair halves get their own
                # tiles instead of slices of one [16,128] tile.
                S_psA = pstat.tile([8, 128], F32, tag="pstat")
                S_psB = pstat.tile([8, 128], F32, tag="pstatB")
                S_psp = {0: S_psA, 1: S_psB}
                t_new = stream.tile([128, 2, SEQ], F16, tag="stream")
                r16p = {}

                def stats_chunk(nt):
                    sl = slice(nt * NTW, (nt + 1) * NTW)
                    if cfg["sq_eng"][nt] == "d":
                        for m in range(2):
                            nc.vector.tensor_tensor(
                                out=sq[:, m, sl], in0=Y[:, m, sl],
                                in1=Y[:, m, sl], op=OP.mult,
                            )
                    for c in range(4):
                        g = nt * 4 + c
                        csl = slice(g * 128, (g + 1) * 128)
                        for m in range(2):
                            nc.tensor.matmul(
                                S_psp[g // 8][:],
                                lhsT=onesI_sb[:, g % 8, :],
                                rhs=sq[:, m, csl],
                                start=(g % 8 == 0 and m == 0),
                                stop=(g % 8 == 7 and m == 1),
                            )

                def rsqrt_pair(p):
                    # NOTE: the bit-trick int ops must read SBUF — DVE reads
                    # of PSUM are value-converted (not bit-cast) on real HW,
                    # so the eps-add TS doubles as the PSUM->SBUF move.
                    v16 = stat.tile([8, 128], F32, tag="v16")
                    y16 = stat.tile([8, 128], F32, tag="y16")
                    t16 = stat.tile([8, 128], F32, tag="t16")
                    r16 = stat.tile([8, 128], F16, tag="r16")
                    r16p[p] = r16
                    if cfg.get("rsqrt_act"):
                        nc.scalar.activation(
                            out=v16[:], in_=S_psp[p][:], func=ACTF.Identity,
                            bias=eps_col[:], scale=1.0,
                        )
                    else:
                        nc.vector.tensor_scalar(
                            out=v16[:], in0=S_psp[p][:], scalar1=LN_EPS,
                            scalar2=None, op0=OP.add,
                        )
                    nc.vector.tensor_tensor(
                        out=y16[:].bitcast(I32), in0=v16[:].bitcast(I32),
                        in1=c_one[0:8, :], op=OP.logical_shift_right,
                    )
                    nc.vector.tensor_tensor(
                        out=y16[:].bitcast(I32), in0=c_magic[0:8, :],
                        in1=y16[:].bitcast(I32), op=OP.subtract,
                    )
                    if cfg.get("rsqrt_act"):
                        nc.scalar.activation(
                            out=t16[:], in_=y16[:], func=ACTF.Square,
                            bias=0.0, scale=1.0,
                        )
                    else:
                        nc.vector.tensor_tensor(
                            out=t16[:], in0=y16[:], in1=y16[:], op=OP.mult
                        )
                    nc.vector.scalar_tensor_tensor(
                        out=t16[:], in0=t16[:], scalar=-0.5, in1=v16[:],
                        op0=OP.mult, op1=OP.mult,
                    )
                    nc.vector.scalar_tensor_tensor(
                        out=r16[:], in0=t16[:], scalar=1.5, in1=y16[:],
                        op0=OP.add, op1=OP.mult,
                    )

                def bcast_tp(nt):
                    # reads only this nt-pair's r16 tile so pair-0 broadcasts
                    # don't wait on pair-1's rsqrt
                    sl = slice(nt * NTW, (nt + 1) * NTW)
                    r16 = r16p[nt // 2]
                    selt = sel_sb
                    ib = pinv.tile([128, NTW], F32, tag="pinv")
                    for c in range(4):
                        g = nt * 4 + c
                        nc.tensor.matmul(
                            ib[:, c * 128 : (c + 1) * 128],
                            lhsT=selt[:, g * 128 : (g + 1) * 128],
                            rhs=r16[:],
                            start=True, stop=True,
                        )
                    if nt in cfg.get("tp_direct", ()):
                        # chain-critical chunk: skip the ACT evac hop and pay
                        # the fp32-rate PSUM read on DVE (runs in a window
                        # where DVE is chain-bound anyway)
                        for m in range(2):
                            nc.vector.scalar_tensor_tensor(
                                out=t_new[:, m, sl], in0=Y[:, m, sl],
                                scalar=0.0, in1=ib[:],
                                op0=OP.bypass, op1=OP.mult,
                            )
                    else:
                        i16 = epool.tile([128, 1, NTW], F16, tag="i16")
                        nc.scalar.activation(
                            out=i16[:, 0], in_=ib[:], func=ACTF.Copy,
                            bias=0.0, scale=1.0,
                        )
                        for m in range(2):
                            nc.vector.tensor_tensor(
                                out=t_new[:, m, sl], in0=Y[:, m, sl],
                                in1=i16[:, 0], op=OP.mult,
                            )

                if cfg["rsqrt_pairs"]:
                    stats_chunk(0)
                    stats_chunk(1)
                    rsqrt_pair(0)
                    if cfg.get("tp_first"):
                        bcast_tp(0)
                        bcast_tp(1)
                        stats_chunk(2)
                        stats_chunk(3)
                        rsqrt_pair(1)
                        bcast_tp(2)
                        bcast_tp(3)
                        return_early = True
                    else:
                        return_early = False
                    if return_early:
                        pass
                    elif True:
                        stats_chunk(2)
                        stats_chunk(3)
                    if return_early:
                        pass
                    elif cfg.get("rsqrtB_early"):
                        rsqrt_pair(1)
                        bcast_tp(0)
                        bcast_tp(1)
                        bcast_tp(2)
                        bcast_tp(3)
                    else:
                        bcast_tp(0)
                        bcast_tp(1)
                        rsqrt_pair(1)
                        bcast_tp(2)
                        bcast_tp(3)
                else:
                    for nt in range(NT):
                        stats_chunk(nt)
                    rsqrt_pair(0)
                    rsqrt_pair(1)
                    for nt in range(NT):
                        bcast_tp(nt)

                if exit_idx is not None:
                    # pooled = sum_l t_new; epilogue-only consumer, so it can
                    # ride the idle Pool engine — except the LAST exit layer,
                    # whose pooled chain would dangle past the end (DVE is
                    # free by then).
                    last = exit_idx == 3
                    peng = nc.vector if (last or not cfg["pool_pooled"]) else nc.gpsimd
                    pc = pacc.tile([128, 2, 2], F32, tag="pacc")
                    pscr = sqpool.tile([128, 2, SEQ], F16, tag="pscr")
                    for m in range(2):
                        for h in range(2):
                            hsl = slice(h * 1024, (h + 1) * 1024)
                            peng.tensor_scalar(
                                out=pscr[:, m, hsl], in0=t_new[:, m, hsl],
                                scalar1=0.0, scalar2=0.0, op0=OP.add,
                                op1=OP.add,
                                accum_out=pc[:, m, h : h + 1],
                            )
                    pooled[exit_idx] = pc
                return t_new

            def emit_head(e):
                pc = pooled[e]  # [128, 2, 2] f32 partial sums
                pr = pacc.tile([128, 2], F32, tag=f"pr{e}")
                nc.vector.tensor_tensor(
                    out=pr[:], in0=pc[:, :, 0], in1=pc[:, :, 1], op=OP.add
                )
                pl = pstat.tile([NUM_CLASSES, 1], F32, tag="pstat")
                for k in range(2):
                    nc.tensor.matmul(
                        pl[:],
                        lhsT=hW_sb[:, k, e],
                        rhs=pr[:, k : k + 1],
                        start=(k == 0),
                        stop=(k == 1),
                    )
                nc.vector.tensor_scalar(
                    out=logits_sb[:, e : e + 1], in0=pl[:],
                    scalar1=hb_sb[:, e : e + 1], scalar2=None, op0=OP.add,
                )

            # layer-0 stream t0 = h0 arrives pre-computed via the first DMA

            # ---------------- layers ----------------
            for i in range(N_LAYERS):
                # conv path: S_inf from t[:, :, :JW], tail correction
                v = vpool.tile([128, 2, SEQ], F16, tag="v")
                sinf = small.tile([128, 2], F32, tag="sinf")
                sscr = small.tile([128, 2, JW], F16, tag="sscr")
                conv = small.tile([128, 2, TAIL], F16, tag="conv")
                ut = small.tile([128, 2, TAIL], F16, tag="ut")
                for m in range(2):
                    nc.vector.scalar_tensor_tensor(
                        out=sscr[:, m], in0=t[:, m, 0:JW], scalar=0.0,
                        in1=W1_sb[:, i, m], op0=OP.bypass, op1=OP.mult,
                        accum_out=sinf[:, m : m + 1],
                    )
                    nc.vector.tensor_scalar(
                        out=conv[:, m], in0=Wt_sb[:, i, m],
                        scalar1=sinf[:, m : m + 1], scalar2=None,
                        op0=OP.mult,
                    )
                    # main gelu over [0, SEQ-TAIL), tail handled separately
                    for g0, g1 in cfg.get("gelu_pieces", ((0, 1024), (1024, SEQ - TAIL))):
                        nc.scalar.activation(
                            out=v[:, m, g0:g1], in_=t[:, m, g0:g1],
                            func=GELU, bias=0.0,
                            scale=Dp_sb[:, i, m : m + 1],
                        )
                    nc.vector.scalar_tensor_tensor(
                        out=ut[:, m], in0=t[:, m, SEQ - TAIL :],
                        scalar=Dp_sb[:, i, m : m + 1], in1=conv[:, m],
                        op0=OP.mult, op1=OP.add,
                    )
                    nc.scalar.activation(
                        out=v[:, m, SEQ - TAIL :], in_=ut[:, m],
                        func=GELU, bias=0.0, scale=1.0,
                    )

                # proj matmuls + join
                Y = ypool.tile([128, 2, SEQ], F16, tag="ypool")
                sq = sqpool.tile([128, 2, SEQ], F16, tag="sq")
                if cfg.get("wide_join"):
                    for pair in range(2):
                        psl = slice(pair * 1024, (pair + 1) * 1024)
                        for m in range(2):
                            ps = pw.tile([128, 2, NTW], F32, tag="pw")
                            for h in range(2):
                                nt = pair * 2 + h
                                sl = slice(nt * NTW, (nt + 1) * NTW)
                                nc.tensor.matmul(
                                    ps[:, h],
                                    lhsT=ident_sb[:],
                                    rhs=t[:, m, sl],
                                    start=True, stop=False,
                                )
                                for k in range(2):
                                    nc.tensor.matmul(
                                        ps[:, h],
                                        lhsT=Pt_sb[:, i, k, m * 128 : (m + 1) * 128],
                                        rhs=v[:, k, sl],
                                        start=False,
                                        stop=(k == 1) and i != 0,
                                    )
                                if i == 0:
                                    nc.tensor.matmul(
                                        ps[:, h],
                                        lhsT=corrW_row[:, m * 128 : (m + 1) * 128],
                                        rhs=x16_row[:, sl],
                                        start=False, stop=True,
                                    )
                            if JOIN_ACT[pair * 2]:
                                nc.scalar.activation(
                                    out=Y[:, m, psl], in_=ps[:], func=ACTF.Identity,
                                    bias=pbt_sb[:, i, m : m + 1], scale=1.0,
                                )
                            else:
                                nc.vector.tensor_scalar(
                                    out=Y[:, m, psl], in0=ps[:],
                                    scalar1=pbt_sb[:, i, m : m + 1], scalar2=None,
                                    op0=OP.add,
                                )
                else:
                    for nt in range(NT):
                        sl = slice(nt * NTW, (nt + 1) * NTW)
                        for m in range(2):
                            ps = pw.tile([128, NTW], F32, tag="pw")
                            nc.tensor.matmul(
                                ps[:],
                                lhsT=ident_sb[:],
                                rhs=t[:, m, sl],
                                start=True, stop=False,
                            )
                            for k in range(2):
                                nc.tensor.matmul(
                                    ps[:],
                                    lhsT=Pt_sb[:, i, k, m * 128 : (m + 1) * 128],
                                    rhs=v[:, k, sl],
                                    start=False,
                                    stop=(k == 1) and i != 0,
                                )
                            if i == 0:
                                # center layer-0's x-dependent channel mean
                                nc.tensor.matmul(
                                    ps[:],
                                    lhsT=corrW_row[:, m * 128 : (m + 1) * 128],
                                    rhs=x16_row[:, sl],
                                    start=False, stop=True,
                                )
                            if JOIN_ACT[nt]:
                                nc.scalar.activation(
                                    out=Y[:, m, sl], in_=ps[:], func=ACTF.Identity,
                                    bias=pbt_sb[:, i, m : m + 1], scale=1.0,
                                )
                            else:
                                nc.vector.tensor_scalar(
                                    out=Y[:, m, sl], in0=ps[:],
                                    scalar1=pbt_sb[:, i, m : m + 1], scalar2=None,
                                    op0=OP.add,
                                )
                            if cfg["sq_eng"][nt] == "a":
                                nc.scalar.activation(
                                    out=sq[:, m, sl], in_=ps[:], func=ACTF.Square,
                                    bias=pbt_sb[:, i, m : m + 1], scale=1.0,
                                )

                exit_idx = EXIT_LAYERS.index(i) if i in EXIT_LAYERS else None
                t = tail(Y, sq, i, exit_idx)

            # ---------------- epilogue: pooled reduce + heads ----------------
            for e in range(4):
                emit_head(e)
            nc.sync.dma_start(out=d_out.ap(), in_=logits_sb[:])

    if split:
        _split_drain_waits(nc, mybir)
    return nc


def _forward_fallback(inputs):
    """Numpy-only exact reference computation (general-inputs path)."""
    import math

    erf = np.vectorize(math.erf)
    x = inputs["x"].astype(np.float32)
    h = x[:, :, 0:1] * inputs["in_W"][None, None, :, 0] + inputs["in_b"]
    logits = []
    head = 0
    Lf = np.arange(SEQ, dtype=np.float32)
    for i in range(N_LAYERS):
        A = 1.0 / (1.0 + np.exp(-inputs["A_params"][i].astype(np.float32)))
        K = (
            inputs["C_params"][i][:, None]
            * (A[:, None] ** Lf[None, :])
            * inputs["B_params"][i][:, None]
        ).astype(np.float32)
        ht = np.swapaxes(h, 1, 2).astype(np.float32)
        out = np.empty_like(ht)
        for b in range(x.shape[0]):
            for d in range(D_MODEL):
                c = np.correlate(
                    np.concatenate([np.zeros(SEQ - 1, np.float32), ht[b, d]]),
                    K[d][::-1],
                    mode="valid",
                )
                out[b, d] = c[:SEQ]
        out = out + inputs["D_params"][i][None, :, None] * ht
        u = np.swapaxes(out, 1, 2)
        vg = u * 0.5 * (1.0 + erf(u / np.sqrt(2.0)))
        w = vg.astype(np.float32) @ inputs["proj_W"][i].T + inputs["proj_b"][i]
        y = h + w
        mu = y.mean(-1, keepdims=True)
        var = y.var(-1, keepdims=True)
        h = (y - mu) / np.sqrt(var + LN_EPS) * inputs["ln_g"][i] + inputs["ln_b"][i]
        if i in EXIT_LAYERS:
            pooled = h.mean(axis=1)
            logits.append(pooled @ inputs["head_W"][head].T + inputs["head_b"][head])
            head += 1
    return np.stack(logits, 0).astype(np.float32)


def _run_device(inputs, trace=False):
    from concourse import bass_utils

    key = "nc"
    if key not in _CACHE:
        _CACHE[key] = _build_nc(sim_safe=False)
    nc = _CACHE[key]

    weights = _host_prep(inputs)
    x = np.asarray(inputs["x"], dtype=np.float32)
    inWf = np.asarray(inputs["in_W"], dtype=np.float64)[:, 0]
    inbf = np.asarray(inputs["in_b"], dtype=np.float64)
    in_maps = []
    for b in range(BATCH):
        m = dict(weights)
        xb = x[b, :, 0].astype(np.float64)
        h0 = xb[:, None] * inWf[None, :] + inbf          # [SEQ, 256]
        m["h0_in"] = np.ascontiguousarray(
            h0.reshape(SEQ, 2, 128).transpose(2, 1, 0)
        ).astype(np.float16)                              # [128, 2, SEQ]
        m["x_row16"] = x[b, :, 0].reshape(1, SEQ).astype(np.float16)
        in_maps.append(m)
    res = bass_utils.run_bass_kernel_spmd(
        nc, in_maps, core_ids=list(range(BATCH)), trace=trace
    )
    out = np.empty((4, BATCH, NUM_CLASSES), dtype=np.float32)
    for b in range(BATCH):
        lg = res.results[b]["logits_out"]
        out[:, b, :] = lg.T
    return out, res


def kernel(**inputs):
    inputs = {k: np.asarray(v) for k, v in inputs.items()}
    maxA = float(1.0 / (1.0 + np.exp(-np.abs(inputs["A_params"]).max())))
    fast = (
        np.all(inputs["ln_g"] == 1.0)
        and np.all(inputs["ln_b"] == 0.0)
        and maxA**TAIL < 1e-30
        and inputs["x"].shape == (BATCH, SEQ, 1)
    )
    if not fast:
        return _forward_fallback(inputs)
    out, _ = _run_device(inputs, trace=False)
    return out


# revision 6
# speedup vs baseline: 1.0046x; 1.0046x over previous
"""DeepSSM Trainium2 kernel v2 (8 NeuronCores, data-parallel over batch).

Same math as v1 (see kernel.py docstring for the conv-collapse derivation)
but restructured around the cost model:

- Stream state is the NORMALIZED activations t (post-LN), kept fp16 in
  SBUF.  This removes v1's phase-A pass entirely: gelu reads t directly
  with the per-partition Dp scale.
- All big elementwise DVE ops use fp16 operands in SBUF, which the DVE
  runs in 4x mode (TensorScalarPtr) or 2x mode (TensorTensor).
- The proj matmuls run in fp16 (1 cyc/row at any N, vs fp32r needing
  N>=256).
- LN variance is computed by 32 tiny fp16 matmuls (ones^T @ sq per
  128-position sub-chunk) writing each sub-chunk's row to its own PSUM
  partition, so var lands already "transposed" as S[16,128] — no DMA
  reshape round-trip.  rsqrt = 5-op bit-trick (1 Newton step) on
  [16,128], then 16 tiny selector matmuls broadcast inv back to
  [128,512] PSUM tiles per chunk.
- The two PSUM-reading joins (Y = ps+pbt+t and t' = Y*inv) are split
  between DVE-direct (fp32-rate STT) and ACT-evac (+per-partition bias)
  followed by a 4x fp16 STT, to balance DVE vs ACT load.
"""

import numpy as np

D_MODEL = 256
N_LAYERS = 8
NUM_CLASSES = 3
BATCH = 8
SEQ = 2048
JW = 256
TAIL = 256
LN_EPS = 1e-5
EXIT_LAYERS = (1, 3, 5, 7)
NT = 4
NTW = SEQ // NT  # 512
NSUB = 16  # 128-position sub-chunks

_CACHE = {}

# engine assignment config (sweepable): per-nt engines
CONFIG = dict(
    join_eng=("d", "d", "a", "a"),   # Y-evac: d=DVE TS, a=ACT Identity
    tp_eng=("d", "d", "d", "d"),     # t' TT engine: d=DVE, p=Pool
    sq_eng=("d", "d", "d", "d"),     # per-nt: a=ACT Square(ps+pbt) from PSUM, d=DVE TT
    gelu_pieces=((0, 512), (512, 1024), (1024, 1792)),
    rsqrt_pairs=True,                 # split rsqrt into two nt-pair halves
    pw_bufs=4,
    h0_late=True,
    pool_pooled=False,                 # exit-layer pooled pass on Pool (last on DVE)
)


def _host_prep(inputs):
    f64 = np.float64
    A = 1.0 / (1.0 + np.exp(-inputs["A_params"].astype(f64)))  # [nl, d]
    lnA = np.log(A)
    CB = inputs["C_params"].astype(f64) * inputs["B_params"].astype(f64)
    j1 = np.arange(JW, dtype=f64)
    lt = (TAIL - 1.0) - np.arange(TAIL, dtype=f64)
    W1 = np.exp(lnA[:, :, None] * j1[None, None, :])            # [nl, d, JW]
    Wt = CB[:, :, None] * np.exp(lnA[:, :, None] * lt[None, None, :])

    def to_chunks(T, dt):  # [nl, d, l] -> [128, nl, 2, l]
        return np.ascontiguousarray(
            T.reshape(N_LAYERS, 2, 128, -1).transpose(2, 0, 1, 3)
        ).astype(dt)

    pW = inputs["proj_W"].astype(f64)                            # [nl, do, di]
    pWc = pW - pW.mean(axis=1, keepdims=True)
    PtT_all = np.ascontiguousarray(
        pWc.transpose(0, 2, 1).reshape(N_LAYERS, 2, 128, D_MODEL).transpose(2, 0, 1, 3)
    ).astype(np.float16)                                          # [128,nl,2,256]

    Dp_all = np.ascontiguousarray(
        inputs["D_params"].reshape(N_LAYERS, 2, 128).transpose(2, 0, 1)
    ).astype(np.float32)
    pb = inputs["proj_b"].astype(f64)
    pbt = pb - pb.mean(axis=1, keepdims=True)
    pbt_all = np.ascontiguousarray(
        pbt.reshape(N_LAYERS, 2, 128).transpose(2, 0, 1)
    ).astype(np.float32)

    # layer-0 stream is RAW h0 = inW*x + in_b; its centering for the LN
    # stats rides in via corrW (x-dependent) and the pbt[0] constant.
    inW = inputs["in_W"][:, 0].astype(f64)
    inb = inputs["in_b"].astype(f64)
    corrW_row = np.full((1, D_MODEL), -inW.mean(), dtype=np.float16)
    pbt_all[:, 0, :] -= np.float32(inb.mean())

    hW = inputs["head_W"].astype(f64) / SEQ                      # [4, nc, d]
    headWT_all = np.ascontiguousarray(
        hW.transpose(2, 0, 1).reshape(2, 128, 4, NUM_CLASSES).transpose(1, 0, 2, 3)
    ).astype(np.float32)                                          # [128,2,4,3]
    headb_all = np.ascontiguousarray(
        inputs["head_b"].astype(np.float32).T.reshape(NUM_CLASSES, 4)
    )

    sel = np.zeros((8, NSUB * 128), np.float16)
    for g in range(NSUB):
        sel[g % 8, g * 128:(g + 1) * 128] = 1.0


    return dict(
        W1_all=to_chunks(W1, np.float16),
        Wt_all=to_chunks(Wt, np.float16),
        PtT_all=PtT_all,
        Dp_all=Dp_all,
        pbt_all=pbt_all,
        corrW_row=corrW_row,
        sel_all=sel,
        ident_in=np.ascontiguousarray(np.eye(128, dtype=np.float16)),
        onesI_in=np.ascontiguousarray(
            np.tile(np.eye(8, dtype=np.float16)[None] / D_MODEL, (128, 1, 1))
        ),
        headWT_all=headWT_all,
        headb_all=headb_all,
    )


def _split_drain_waits(nc, mybir, maxw=1):
    """Hoist excess sync waits onto same-engine NOPs (walrus ISA limit)."""
    for f in nc.m.functions:
        for blk in f.blocks:
            insts = list(blk.instructions)
            changed = False
            new_list = []
            for ins in insts:
                w = (
                    list(ins.sync_info.on_wait)
                    if ins.sync_info and ins.sync_info.on_wait
                    else []
                )
                if len(w) > maxw:
                    changed = True
                    extra, keep = w[:-maxw], w[-maxw:]
                    for j in range(0, len(extra), maxw):
                        nop = mybir.InstNoOp(
                            name=f"{ins.name}-wsplit{j}", ins=[], outs=[]
                        )
                        nop.engine = ins.engine
                        nop.sync_info = mybir.SyncInfo(
                            on_wait=extra[j : j + maxw], on_update=[]
                        )
                        new_list.append(nop)
                    ins.sync_info.on_wait = keep
                new_list.append(ins)
            if changed:
                blk.instructions = new_list


def _build_nc(sim_safe=False, split=True):
    import concourse.bass as bass
    import concourse.tile as tile
    import concourse.mybir as mybir

    F32 = mybir.dt.float32
    F32R = mybir.dt.float32r
    F16 = mybir.dt.float16
    I32 = mybir.dt.int32
    OP = mybir.AluOpType
    ACTF = mybir.ActivationFunctionType
    GELU = ACTF.Sigmoid if sim_safe else ACTF.Gelu

    nc = bass.Bass("TRN2", target_bir_lowering=False, debug=False)

    d_h0 = nc.dram_tensor("h0_in", [128, 2, SEQ], F16, kind="ExternalInput")
    d_W1 = nc.dram_tensor("W1_all", [128, N_LAYERS, 2, JW], F16, kind="ExternalInput")
    d_Wt = nc.dram_tensor("Wt_all", [128, N_LAYERS, 2, TAIL], F16, kind="ExternalInput")
    d_Pt = nc.dram_tensor("PtT_all", [128, N_LAYERS, 2, D_MODEL], F16, kind="ExternalInput")
    d_Dp = nc.dram_tensor("Dp_all", [128, N_LAYERS, 2], F32, kind="ExternalInput")
    d_pbt = nc.dram_tensor("pbt_all", [128, N_LAYERS, 2], F32, kind="ExternalInput")
    d_corrW = nc.dram_tensor("corrW_row", [1, D_MODEL], F16, kind="ExternalInput")
    d_x16 = nc.dram_tensor("x_row16", [1, SEQ], F16, kind="ExternalInput")
    d_sel = nc.dram_tensor("sel_all", [8, NSUB * 128], F16, kind="ExternalInput")
    d_onesI = nc.dram_tensor("onesI_in", [128, 8, 8], F16, kind="ExternalInput")
    d_ident = nc.dram_tensor("ident_in", [128, 128], F16, kind="ExternalInput")
    d_hW = nc.dram_tensor("headWT_all", [128, 2, 4, NUM_CLASSES], F32, kind="ExternalInput")
    d_hb = nc.dram_tensor("headb_all", [NUM_CLASSES, 4], F32, kind="ExternalInput")
    d_out = nc.dram_tensor("logits_out", [NUM_CLASSES, 4], F32, kind="ExternalOutput")

    with tile.TileContext(nc) as tc:
        from contextlib import ExitStack

        ctx = ExitStack()
        with ctx:
            const = ctx.enter_context(tc.tile_pool(name="const", bufs=1))
            stream = ctx.enter_context(tc.tile_pool(name="stream", bufs=CONFIG.get("stream_bufs", 3)))
            ypool = ctx.enter_context(tc.tile_pool(name="ypool", bufs=CONFIG.get("y_bufs", 2)))
            vpool = ctx.enter_context(tc.tile_pool(name="vpool", bufs=CONFIG.get("v_bufs", 2)))
            sqpool = ctx.enter_context(tc.tile_pool(name="sqpool", bufs=CONFIG.get("sq_bufs", 2)))
            epool = ctx.enter_context(tc.tile_pool(name="epool", bufs=2))
            small = ctx.enter_context(tc.tile_pool(name="small", bufs=2))
            stat = ctx.enter_context(tc.tile_pool(name="stat", bufs=2))
            pacc = ctx.enter_context(tc.tile_pool(name="pacc", bufs=5))
            pw = ctx.enter_context(tc.tile_pool(name="pw", bufs=CONFIG.get("pw_bufs", 3), space="PSUM"))
            pstat = ctx.enter_context(tc.tile_pool(name="pstat", bufs=1, space="PSUM"))
            pinv = ctx.enter_context(tc.tile_pool(name="pinv", bufs=2, space="PSUM"))

            # ---- constants / weights to SBUF ----
            # SP issues DMAs in-order: layer-0's working set first, cold
            # layers and head tables last.
            t = stream.tile([128, 2, SEQ], F16, tag="stream")
            nc.sync.dma_start(out=t[:, :, 0:NTW], in_=d_h0.ap()[:, :, 0:NTW])
            Dp_sb = const.tile([128, N_LAYERS, 2], F32)
            nc.sync.dma_start(out=Dp_sb[:], in_=d_Dp.ap())
            ident_sb = const.tile([128, 128], F16)
            nc.sync.dma_start(out=ident_sb[:], in_=d_ident.ap())
            Pt_sb = const.tile([128, N_LAYERS, 2, D_MODEL], F16)
            W1_sb = const.tile([128, N_LAYERS, 2, JW], F16)
            Wt_sb = const.tile([128, N_LAYERS, 2, TAIL], F16)
            nc.sync.dma_start(out=W1_sb[:, 0], in_=d_W1.ap()[:, 0])
            nc.sync.dma_start(out=t[:, :, NTW : 2 * NTW], in_=d_h0.ap()[:, :, NTW : 2 * NTW])
            nc.sync.dma_start(out=Wt_sb[:, 0], in_=d_Wt.ap()[:, 0])
            if CONFIG.get("h0_late"):
                nc.sync.dma_start(out=t[:, :, 2 * NTW : 3 * NTW], in_=d_h0.ap()[:, :, 2 * NTW : 3 * NTW])
                nc.sync.dma_start(out=Pt_sb[:, 0], in_=d_Pt.ap()[:, 0])
                nc.sync.dma_start(out=t[:, :, 3 * NTW :], in_=d_h0.ap()[:, :, 3 * NTW :])
            else:
                nc.sync.dma_start(out=Pt_sb[:, 0], in_=d_Pt.ap()[:, 0])
                nc.sync.dma_start(out=t[:, :, 2 * NTW : 3 * NTW], in_=d_h0.ap()[:, :, 2 * NTW : 3 * NTW])
                nc.sync.dma_start(out=t[:, :, 3 * NTW :], in_=d_h0.ap()[:, :, 3 * NTW :])
            pbt_sb = const.tile([128, N_LAYERS, 2], F32)
            nc.sync.dma_start(out=pbt_sb[:], in_=d_pbt.ap())
            x16_row = const.tile([1, SEQ], F16)
            nc.sync.dma_start(out=x16_row[:], in_=d_x16.ap())
            corrW_row = const.tile([1, D_MODEL], F16)
            nc.sync.dma_start(out=corrW_row[:], in_=d_corrW.ap())
            onesI_sb = const.tile([128, 8, 8], F16)
            nc.sync.dma_start(out=onesI_sb[:], in_=d_onesI.ap())
            sel_sb = const.tile([8, NSUB * 128], F16)
            nc.sync.dma_start(out=sel_sb[:], in_=d_sel.ap())
            for i in range(1, N_LAYERS):
                nc.sync.dma_start(out=W1_sb[:, i], in_=d_W1.ap()[:, i])
                nc.sync.dma_start(out=Wt_sb[:, i], in_=d_Wt.ap()[:, i])
                nc.sync.dma_start(out=Pt_sb[:, i], in_=d_Pt.ap()[:, i])
            hW_sb = const.tile([128, 2, 4, NUM_CLASSES], F32)
            nc.sync.dma_start(out=hW_sb[:], in_=d_hW.ap())
            hb_sb = const.tile([NUM_CLASSES, 4], F32)
            nc.sync.dma_start(out=hb_sb[:], in_=d_hb.ap())
            c_one = const.tile([NSUB, 128], I32)
            nc.vector.memset(c_one, 1)
            eps_col = const.tile([8, 1], F32)
            nc.vector.memset(eps_col, LN_EPS)
            c_magic = const.tile([NSUB, 128], I32)
            nc.vector.memset(c_magic, 0x5F3759DF)

            logits_sb = const.tile([NUM_CLASSES, 4], F32)
            pooled = {}

            # ---------------- shared tail: Y -> t_next ----------------
            cfg = CONFIG
            JOIN_ACT = tuple(e == "a" for e in cfg["join_eng"])

            def tail(Y, sq, li, exit_idx):
                """Y fp16 [128,2,SEQ] (centered pre-LN) -> t fp16 (normalized).

                Emitted per nt-chunk (sq+stats right after the producing
                joins), with the rsqrt split into two nt-pair halves so the
                second half's statistics overlap the first half's broadcast
                and t' work."""
                # two [8,128] stat tiles: HW requires partition offset 0 (or
                # 32/64) on every access, so the pair halves get their own
                # tiles instead of slices of one [16,128] tile.
                S_psA = pstat.tile([8, 128], F32, tag="pstat")
                S_psB = pstat.tile([8, 128], F32, tag="pstatB")
                S_psp = {0: S_psA, 1: S_psB}
                t_new = stream.tile([128, 2, SEQ], F16, tag="stream")
                r16p = {}

                def stats_chunk(nt):
                    sl = slice(nt * NTW, (nt + 1) * NTW)
                    if cfg["sq_eng"][nt] == "d":
                        for m in range(2):
                            nc.vector.tensor_tensor(
                                out=sq[:, m, sl], in0=Y[:, m, sl],
                                in1=Y[:, m, sl], op=OP.mult,
                            )
                    for c in range(4):
                        g = nt * 4 + c
                        csl = slice(g * 128, (g + 1) * 128)
                        for m in range(2):
                            nc.tensor.matmul(
                                S_psp[g // 8][:],
                                lhsT=onesI_sb[:, g % 8, :],
                                rhs=sq[:, m, csl],
                                start=(g % 8 == 0 and m == 0),
                                stop=(g % 8 == 7 and m == 1),
                            )

                def rsqrt_pair(p):
                    # NOTE: the bit-trick int ops must read SBUF — DVE reads
                    # of PSUM are value-converted (not bit-cast) on real HW,
                    # so the eps-add TS doubles as the PSUM->SBUF move.
                    v16 = stat.tile([8, 128], F32, tag="v16")
                    y16 = stat.tile([8, 128], F32, tag="y16")
                    t16 = stat.tile([8, 128], F32, tag="t16")
                    r16 = stat.tile([8, 128], F16, tag="r16")
                    r16p[p] = r16
                    if cfg.get("rsqrt_act"):
                        nc.scalar.activation(
                            out=v16[:], in_=S_psp[p][:], func=ACTF.Identity,
                            bias=eps_col[:], scale=1.0,
                        )
                    else:
                        nc.vector.tensor_scalar(
                            out=v16[:], in0=S_psp[p][:], scalar1=LN_EPS,
                            scalar2=None, op0=OP.add,
                        )
                    nc.vector.tensor_tensor(
                        out=y16[:].bitcast(I32), in0=v16[:].bitcast(I32),
                        in1=c_one[0:8, :], op=OP.logical_shift_right,
                    )
                    nc.vector.tensor_tensor(
                        out=y16[:].bitcast(I32), in0=c_magic[0:8, :],
                        in1=y16[:].bitcast(I32), op=OP.subtract,
                    )
                    if cfg.get("rsqrt_act"):
                        nc.scalar.activation(
                            out=t16[:], in_=y16[:], func=ACTF.Square,
                            bias=0.0, scale=1.0,
                        )
                    else:
                        nc.vector.tensor_tensor(
                            out=t16[:], in0=y16[:], in1=y16[:], op=OP.mult
                        )
                    nc.vector.scalar_tensor_tensor(
                        out=t16[:], in0=t16[:], scalar=-0.5, in1=v16[:],
                        op0=OP.mult, op1=OP.mult,
                    )
                    nc.vector.scalar_tensor_tensor(
                        out=r16[:], in0=t16[:], scalar=1.5, in1=y16[:],
                        op0=OP.add, op1=OP.mult,
                    )

                def bcast_tp(nt):
                    # reads only this nt-pair's r16 tile so pair-0 broadcasts
                    # don't wait on pair-1's rsqrt
                    sl = slice(nt * NTW, (nt + 1) * NTW)
                    r16 = r16p[nt // 2]
                    selt = sel_sb
                    ib = pinv.tile([128, NTW], F32, tag="pinv")
                    for c in range(4):
                        g = nt * 4 + c
                        nc.tensor.matmul(
                            ib[:, c * 128 : (c + 1) * 128],
                            lhsT=selt[:, g * 128 : (g + 1) * 128],
                            rhs=r16[:],
                            start=True, stop=True,
                        )
                    if nt in cfg.get("tp_direct", ()):
                        # chain-critical chunk: skip the ACT evac hop and pay
                        # the fp32-rate PSUM read on DVE (runs in a window
                        # where DVE is chain-bound anyway)
                        for m in range(2):
                            nc.vector.scalar_tensor_tensor(
                                out=t_new[:, m, sl], in0=Y[:, m, sl],
                                scalar=0.0, in1=ib[:],
                                op0=OP.bypass, op1=OP.mult,
                            )
                    else:
                        i16 = epool.tile([128, 1, NTW], F16, tag="i16")
                        nc.scalar.activation(
                            out=i16[:, 0], in_=ib[:], func=ACTF.Copy,
                            bias=0.0, scale=1.0,
                        )
                        for m in range(2):
                            nc.vector.tensor_tensor(
                                out=t_new[:, m, sl], in0=Y[:, m, sl],
                                in1=i16[:, 0], op=OP.mult,
                            )

                if cfg["rsqrt_pairs"]:
                    stats_chunk(0)
                    stats_chunk(1)
                    rsqrt_pair(0)
                    if cfg.get("tp_first"):
                        bcast_tp(0)
                        bcast_tp(1)
                        stats_chunk(2)
                        stats_chunk(3)
                        rsqrt_pair(1)
                        bcast_tp(2)
                        bcast_tp(3)
                        return_early = True
                    else:
                        return_early = False
                    if return_early:
                        pass
                    elif True:
                        stats_chunk(2)
                        stats_chunk(3)
                    if return_early:
                        pass
                    elif cfg.get("rsqrtB_early"):
                        rsqrt_pair(1)
                        bcast_tp(0)
                        bcast_tp(1)
                        bcast_tp(2)
                        bcast_tp(3)
                    else:
                        bcast_tp(0)
                        bcast_tp(1)
                        rsqrt_pair(1)
                        bcast_tp(2)
                        bcast_tp(3)
                else:
                    for nt in range(NT):
                        stats_chunk(nt)
                    rsqrt_pair(0)
                    rsqrt_pair(1)
                    for nt in range(NT):
                        bcast_tp(nt)

                if exit_idx is not None:
                    # pooled = sum_l t_new; epilogue-only consumer, so it can
                    # ride the idle Pool engine — except the LAST exit layer,
                    # whose pooled chain would dangle past the end (DVE is
                    # free by then).
                    last = exit_idx == 3
                    peng = nc.vector if (last or not cfg["pool_pooled"]) else nc.gpsimd
                    pc = pacc.tile([128, 2, 2], F32, tag="pacc")
                    pscr = sqpool.tile([128, 2, SEQ], F16, tag="pscr")
                    for m in range(2):
                        for h in range(2):
                            hsl = slice(h * 1024, (h + 1) * 1024)
                            peng.tensor_scalar(
                                out=pscr[:, m, hsl], in0=t_new[:, m, hsl],
                                scalar1=0.0, scalar2=0.0, op0=OP.add,
                                op1=OP.add,
                                accum_out=pc[:, m, h : h + 1],
                            )
                    pooled[exit_idx] = pc
                return t_new

            def emit_head(e):
                pc = pooled[e]  # [128, 2, 2] f32 partial sums
                pr = pacc.tile([128, 2], F32, tag=f"pr{e}")
                nc.vector.tensor_tensor(
                    out=pr[:], in0=pc[:, :, 0], in1=pc[:, :, 1], op=OP.add
                )
                pl = pstat.tile([NUM_CLASSES, 1], F32, tag="pstat")
                for k in range(2):
                    nc.tensor.matmul(
                        pl[:],
                        lhsT=hW_sb[:, k, e],
                        rhs=pr[:, k : k + 1],
                        start=(k == 0),
                        stop=(k == 1),
                    )
                nc.vector.tensor_scalar(
                    out=logits_sb[:, e : e + 1], in0=pl[:],
                    scalar1=hb_sb[:, e : e + 1], scalar2=None, op0=OP.add,
                )

            # layer-0 stream t0 = h0 arrives pre-computed via the first DMA

            # ---------------- layers ----------------
            for i in range(N_LAYERS):
                # conv path: S_inf from t[:, :, :JW], tail correction
                v = vpool.tile([128, 2, SEQ], F16, tag="v")
                sinf = small.tile([128, 2], F32, tag="sinf")
                sscr = small.tile([128, 2, JW], F16, tag="sscr")
                conv = small.tile([128, 2, TAIL], F16, tag="conv")
                ut = small.tile([128, 2, TAIL], F16, tag="ut")
                for m in range(2):
                    nc.vector.scalar_tensor_tensor(
                        out=sscr[:, m], in0=t[:, m, 0:JW], scalar=0.0,
                        in1=W1_sb[:, i, m], op0=OP.bypass, op1=OP.mult,
                        accum_out=sinf[:, m : m + 1],
                    )
                    nc.vector.tensor_scalar(
                        out=conv[:, m], in0=Wt_sb[:, i, m],
                        scalar1=sinf[:, m : m + 1], scalar2=None,
                        op0=OP.mult,
                    )
                    # main gelu over [0, SEQ-TAIL), tail handled separately
                    for g0, g1 in cfg.get("gelu_pieces", ((0, 1024), (1024, SEQ - TAIL))):
                        nc.scalar.activation(
                            out=v[:, m, g0:g1], in_=t[:, m, g0:g1],
                            func=GELU, bias=0.0,
                            scale=Dp_sb[:, i, m : m + 1],
                        )
                    nc.vector.scalar_tensor_tensor(
                        out=ut[:, m], in0=t[:, m, SEQ - TAIL :],
                        scalar=Dp_sb[:, i, m : m + 1], in1=conv[:, m],
                        op0=OP.mult, op1=OP.add,
                    )
                    nc.scalar.activation(
                        out=v[:, m, SEQ - TAIL :], in_=ut[:, m],
                        func=GELU, bias=0.0, scale=1.0,
                    )

                # proj matmuls + join
                Y = ypool.tile([128, 2, SEQ], F16, tag="ypool")
                sq = sqpool.tile([128, 2, SEQ], F16, tag="sq")
                if cfg.get("wide_join"):
                    for pair in range(2):
                        psl = slice(pair * 1024, (pair + 1) * 1024)
                        for m in range(2):
                            ps = pw.tile([128, 2, NTW], F32, tag="pw")
                            for h in range(2):
                                nt = pair * 2 + h
                                sl = slice(nt * NTW, (nt + 1) * NTW)
                                nc.tensor.matmul(
                                    ps[:, h],
                                    lhsT=ident_sb[:],
                                    rhs=t[:, m, sl],
                                    start=True, stop=False,
                                )
                                for k in range(2):
                                    nc.tensor.matmul(
                                        ps[:, h],
                                        lhsT=Pt_sb[:, i, k, m * 128 : (m + 1) * 128],
                                        rhs=v[:, k, sl],
                                        start=False,
                                        stop=(k == 1) and i != 0,
                                    )
                                if i == 0:
                                    nc.tensor.matmul(
                                        ps[:, h],
                                        lhsT=corrW_row[:, m * 128 : (m + 1) * 128],
                                        rhs=x16_row[:, sl],
                                        start=False, stop=True,
                                    )
                            if JOIN_ACT[pair * 2]:
                                nc.scalar.activation(
                                    out=Y[:, m, psl], in_=ps[:], func=ACTF.Identity,
                                    bias=pbt_sb[:, i, m : m + 1], scale=1.0,
                                )
                            else:
                                nc.vector.tensor_scalar(
                                    out=Y[:, m, psl], in0=ps[:],
                                    scalar1=pbt_sb[:, i, m : m + 1], scalar2=None,
                                    op0=OP.add,
                                )
                else:
                    for nt in range(NT):
                        sl = slice(nt * NTW, (nt + 1) * NTW)
                        for m in range(2):
                            ps = pw.tile([128, NTW], F32, tag="pw")
                            nc.tensor.matmul(
                                ps[:],
                                lhsT=ident_sb[:],
                                rhs=t[:, m, sl],
                                start=True, stop=False,
                            )
                            for k in range(2):
                                nc.tensor.matmul(
                                    ps[:],
                                    lhsT=Pt_sb[:, i, k, m * 128 : (m + 1) * 128],
                                    rhs=v[:, k, sl],
                                    start=False,
                                    stop=(k == 1) and i != 0,
                                )
                            if i == 0:
                                # center layer-0's x-dependent channel mean
                                nc.tensor.matmul(
                                    ps[:],
                                    lhsT=corrW_row[:, m * 128 : (m + 1) * 128],
                                    rhs=x16_row[:, sl],
                                    start=False, stop=True,
                                )
                            if JOIN_ACT[nt]:
                                nc.scalar.activation(
                                    out=Y[:, m, sl], in_=ps[:], func=ACTF.Identity,
                                    bias=pbt_sb[:, i, m : m + 1], scale=1.0,
                                )
                            else:
                                nc.vector.tensor_scalar(
                                    out=Y[:, m, sl], in0=ps[:],
                                    scalar1=pbt_sb[:, i, m : m + 1], scalar2=None,
                                    op0=OP.add,
                                )
                            if cfg["sq_eng"][nt] == "a":
                                nc.scalar.activation(
                                    out=sq[:, m, sl], in_=ps[:], func=ACTF.Square,
                                    bias=pbt_sb[:, i, m : m + 1], scale=1.0,
                                )

                exit_idx = EXIT_LAYERS.index(i) if i in EXIT_LAYERS else None
                t = tail(Y, sq, i, exit_idx)

            # ---------------- epilogue: pooled reduce + heads ----------------
            for e in range(4):
                emit_head(e)
            nc.sync.dma_start(out=d_out.ap(), in_=logits_sb[:])

    if split:
        _split_drain_waits(nc, mybir)
    return nc


def _forward_fallback(inputs):
    """Numpy-only exact reference computation (general-inputs path)."""
    import math

    erf = np.vectorize(math.erf)
    x = inputs["x"].astype(np.float32)
    h = x[:, :, 0:1] * inputs["in_W"][None, None, :, 0] + inputs["in_b"]
    logits = []
    head = 0
    Lf = np.arange(SEQ, dtype=np.float32)
    for i in range(N_LAYERS):
        A = 1.0 / (1.0 + np.exp(-inputs["A_params"][i].astype(np.float32)))
        K = (
            inputs["C_params"][i][:, None]
            * (A[:, None] ** Lf[None, :])
            * inputs["B_params"][i][:, None]
        ).astype(np.float32)
        ht = np.swapaxes(h, 1, 2).astype(np.float32)
        out = np.empty_like(ht)
        for b in range(x.shape[0]):
            for d in range(D_MODEL):
                c = np.correlate(
                    np.concatenate([np.zeros(SEQ - 1, np.float32), ht[b, d]]),
                    K[d][::-1],
                    mode="valid",
                )
                out[b, d] = c[:SEQ]
        out = out + inputs["D_params"][i][None, :, None] * ht
        u = np.swapaxes(out, 1, 2)
        vg = u * 0.5 * (1.0 + erf(u / np.sqrt(2.0)))
        w = vg.astype(np.float32) @ inputs["proj_W"][i].T + inputs["proj_b"][i]
        y = h + w
        mu = y.mean(-1, keepdims=True)
        var = y.var(-1, keepdims=True)
        h = (y - mu) / np.sqrt(var + LN_EPS) * inputs["ln_g"][i] + inputs["ln_b"][i]
        if i in EXIT_LAYERS:
            pooled = h.mean(axis=1)
            logits.append(pooled @ inputs["head_W"][head].T + inputs["head_b"][head])
            head += 1
    return np.stack(logits, 0).astype(np.float32)


def _run_device(inputs, trace=False):
    from concourse import bass_utils

    key = "nc"
    if key not in _CACHE:
        _CACHE[key] = _build_nc(sim_safe=False)
    nc = _CACHE[key]

    weights = _host_prep(inputs)
    x = np.asarray(inputs["x"], dtype=np.float32)
    inWf = np.asarray(inputs["in_W"], dtype=np.float64)[:, 0]
    inbf = np.asarray(inputs["in_b"], dtype=np.float64)
    in_maps = []
    for b in range(BATCH):
        m = dict(weights)
        xb = x[b, :, 0].astype(np.float64)
        h0 = xb[:, None] * inWf[None, :] + inbf          # [SEQ, 256]
        m["h0_in"] = np.ascontiguousarray(
            h0.reshape(SEQ, 2, 128).transpose(2, 1, 0)
        ).astype(np.float16)                              # [128, 2, SEQ]
        m["x_row16"] = x[b, :, 0].reshape(1, SEQ).astype(np.float16)
        in_maps.append(m)
    res = bass_utils.run_bass_kernel_spmd(
        nc, in_maps, core_ids=list(range(BATCH)), trace=trace
    )
    out = np.empty((4, BATCH, NUM_CLASSES), dtype=np.float32)
    for b in range(BATCH):
        lg = res.results[b]["logits_out"]
        out[:, b, :] = lg.T
    return out, res


def kernel(**inputs):
    inputs = {k: np.asarray(v) for k, v in inputs.items()}
    maxA = float(1.0 / (1.0 + np.exp(-np.abs(inputs["A_params"]).max())))
    fast = (
        np.all(inputs["ln_g"] == 1.0)
        and np.all(inputs["ln_b"] == 0.0)
        and maxA**TAIL < 1e-30
        and inputs["x"].shape == (BATCH, SEQ, 1)
    )
    if not fast:
        return _forward_fallback(inputs)
    out, _ = _run_device(inputs, trace=False)
    return out
